# revision 9
# baseline (speedup 1.0000x reference)
"""Causal self-attention with RoPE on 8 Trainium2 NeuronCores.

Sharding: 8 cores = 2 batches x 4 head-groups (4 heads each). Each core
computes QKV for its heads, full causal attention, and a partial output
projection against its slice of w_proj rows; the host sums the 4 partials
per batch element.

All matmul operands use float32r (rounded fp32): full PE rate at N>=256
with ~1.5e-4 relative error (measured), vs 4x slower for plain fp32.
"""
import os

import numpy as np

import concourse.bass as bass
import concourse.mybir as mybir
import concourse.tile as tile
from concourse import bacc
from concourse.bass_utils import run_bass_kernel_spmd

# Problem shape (hardcoded per harness contract).
B, T, C, NH = 2, 2048, 1024, 16
HD = C // NH          # 64
HPC = NH // 4         # 4 heads per core
N_CORES = 8
ROPE_BASE = 10000.0
NEG = -1.0e30

F32 = mybir.dt.float32
F32R = mybir.dt.float32r

_CACHE = {}


def _rope_tables_T():
    """cos/sin tables transposed to [HD, T], duplicated to 128 partitions
    (two 64-row head blocks), with the rotate-half sign folded into sin."""
    inv_freq = 1.0 / (ROPE_BASE ** (np.arange(0, HD, 2, dtype=np.float32) / HD))
    t = np.arange(T, dtype=np.float32)
    freqs = np.outer(t, inv_freq).astype(np.float32)      # [T, 32]
    emb = np.concatenate([freqs, freqs], axis=-1)         # [T, 64]
    cosT = np.cos(emb).T.astype(np.float32)               # [64, T]
    sinT = np.sin(emb).T.astype(np.float32)
    sinm = sinT.copy()
    sinm[: HD // 2] = -sinm[: HD // 2]
    cos2 = np.concatenate([cosT, cosT], axis=0)           # [128, T]
    sinm2 = np.concatenate([sinm, sinm], axis=0)
    return np.ascontiguousarray(cos2), np.ascontiguousarray(sinm2)


def _mask_tiles():
    """maskneg [128, 4, 512]: pattern p for diagonal k-tile j = 4c+p.
    Block coords: k_local in [0,128), q_local in [0,512).
    Allowed iff q_local >= k_local + 128p; disallowed -> NEG."""
    k_l = np.arange(128)[:, None]
    q_l = np.arange(512)[None, :]
    m = np.zeros((128, 4, 512), dtype=np.float32)
    for p in range(4):
        allowed = q_l >= (k_l + 128 * p)
        m[:, p, :] = np.where(allowed, 0.0, NEG)
    return m


def build_nc():
    nc = bacc.Bacc(None, target_bir_lowering=False)

    xT = nc.dram_tensor("xT", [C, T], F32R, kind="ExternalInput")
    wqk = nc.dram_tensor("wqk", [C, 8 * HD], F32R, kind="ExternalInput")
    wv = nc.dram_tensor("wv", [C, 4 * HD], F32R, kind="ExternalInput")
    wp = nc.dram_tensor("wp", [4 * HD, C], F32R, kind="ExternalInput")
    cos2_d = nc.dram_tensor("cos2", [128, T], F32, kind="ExternalInput")
    sinm2_d = nc.dram_tensor("sinm2", [128, T], F32, kind="ExternalInput")
    maskneg_d = nc.dram_tensor("maskneg", [128, 4, 512], F32R, kind="ExternalInput")
    ident_d = nc.dram_tensor("ident", [128, 128], F32R, kind="ExternalInput")
    sel4_d = nc.dram_tensor("sel4", [4, 4 * HD], F32R, kind="ExternalInput")
    outp = nc.dram_tensor("outp", [T, C], F32, kind="ExternalOutput")

    NT = T // 128    # 16 k-tiles
    NQ = T // 512    # 4 q-chunks

    with tile.TileContext(nc) as tc:
        with (
            tc.tile_pool(name="persist", bufs=1) as persist,
            tc.tile_pool(name="consts", bufs=1) as consts,
        ):
            # ---- persistent tiles (across phases) ----
            qk_packed = [
                persist.tile([128, T], F32R, name=f"qkp{w}", tag=f"qkp{w}")
                for w in range(4)
            ]
            vtil = persist.tile([128, NT, 4, HD + 1], F32R, name="vtil")
            ynorm = [
                persist.tile([128, T], F32R, name=f"ynorm{g}", tag=f"ynorm{g}")
                for g in range(2)
            ]
            ident_sb = consts.tile([128, 128], F32R, name="ident_sb")
            nc.sync.dma_start(out=ident_sb, in_=ident_d[:, :])
            maskneg_sb = consts.tile([128, 4, 512], F32R, name="maskneg_sb")
            nc.sync.dma_start(out=maskneg_sb, in_=maskneg_d[:, :, :])
            sel4_sb = consts.tile([4, 4 * HD], F32R, name="sel4_sb")
            nc.sync.dma_start(out=sel4_sb, in_=sel4_d[:, :])

            # ================= Phase 1: QKV + RoPE =================
            with (
                tc.tile_pool(name="p1", bufs=1) as p1,
                tc.tile_pool(name="p1w", bufs=2) as p1w,
                tc.tile_pool(name="p1tmp", bufs=1) as p1tmp,
                tc.tile_pool(name="ps1", bufs=4, space="PSUM") as ps1,
                tc.tile_pool(name="ps1v", bufs=2, space="PSUM") as ps1v,
            ):
                xT_sb = p1.tile([128, C // 128, T], F32R, name="xT_sb")
                xT_r = xT.rearrange("(co p) t -> p co t", p=128)
                for c in range(C // 128):
                    nc.sync.dma_start(out=xT_sb[:, c, :], in_=xT_r[:, c, :])
                cos2_sb = p1.tile([128, T], F32, name="cos2_sb")
                nc.sync.dma_start(out=cos2_sb, in_=cos2_d[:, :])
                sinm2_sb = p1.tile([128, T], F32, name="sinm2_sb")
                nc.sync.dma_start(out=sinm2_sb, in_=sinm2_d[:, :])
                wv_sb = p1.tile([128, C // 128, 4 * HD], F32R, name="wv_sb")
                nc.sync.dma_start(
                    out=wv_sb, in_=wv.rearrange("(co p) n -> p co n", p=128)
                )
                wqk_r = wqk.rearrange("(co p) n -> p co n", p=128)

                # --- Q^T / K^T packed two heads per 128 partitions ---
                for w in range(4):
                    wqk_sb = p1w.tile([128, C // 128, 128], F32R, name="wqk_sb",
                                      tag="wqk_sb")
                    nc.sync.dma_start(
                        out=wqk_sb, in_=wqk_r[:, :, w * 128:(w + 1) * 128]
                    )
                    raw = p1tmp.tile([128, T], F32, name="raw", tag="raw")
                    for q in range(NQ):
                        ps = ps1.tile([128, 512], F32, name="ps_qk", tag="ps_qk")
                        for c in range(C // 128):
                            nc.tensor.matmul(
                                ps,
                                lhsT=wqk_sb[:, c, :],
                                rhs=xT_sb[:, c, q * 512:(q + 1) * 512],
                                start=(c == 0),
                                stop=(c == C // 128 - 1),
                            )
                        nc.scalar.copy(out=raw[:, q * 512:(q + 1) * 512], in_=ps)
                    # rotate-half source: swap 32-partition blocks within each
                    # 64-row head block (cross-partition -> DMA).
                    swp = p1tmp.tile([128, T], F32, name="swp", tag="swp")
                    for blk in range(4):
                        src = (blk ^ 1) * 32
                        nc.sync.dma_start(
                            out=swp[blk * 32:(blk + 1) * 32, :],
                            in_=raw[src:src + 32, :],
                        )
                    nc.vector.tensor_mul(raw, raw, cos2_sb)
                    nc.vector.tensor_mul(swp, swp, sinm2_sb)
                    nc.vector.tensor_add(qk_packed[w], raw, swp)

                # --- V (t-major) + ones column ---
                ones64 = p1.tile([128, NT * 4], F32, name="ones64")
                nc.vector.memset(ones64, 1.0)
                nc.vector.tensor_copy(
                    out=vtil[:, :, :, HD:HD + 1],
                    in_=ones64.rearrange("p (a b) -> p a b", a=NT).unsqueeze(-1),
                )
                for tt in range(NT):
                    psv = ps1v.tile([128, 4 * HD], F32, name="psv", tag="psv")
                    for c in range(C // 128):
                        nc.tensor.matmul(
                            psv,
                            lhsT=xT_sb[:, c, tt * 128:(tt + 1) * 128],
                            rhs=wv_sb[:, c, :],
                            start=(c == 0),
                            stop=(c == C // 128 - 1),
                        )
                    nc.vector.tensor_copy(
                        out=vtil[:, tt, :, 0:HD],
                        in_=psv.rearrange("p (h d) -> p h d", h=4),
                    )

            # ================= Phase 2: attention =================
            with (
                tc.tile_pool(name="p2", bufs=1) as p2,
                tc.tile_pool(name="p2e", bufs=3) as p2e,
                tc.tile_pool(name="p2d", bufs=2) as p2d,
                tc.tile_pool(name="psS", bufs=2, space="PSUM") as psS_pool,
                tc.tile_pool(name="psY", bufs=1, space="PSUM") as psY_pool,
                tc.tile_pool(name="psB", bufs=1, space="PSUM") as psB_pool,
            ):
                inv_sqrt_hd = float(1.0 / np.sqrt(HD))
                for g in range(2):          # head-pair (pack) index
                    ytils = []
                    denoms = []
                    for hh in range(2):
                        ytils.append(
                            p2.tile([HD + 1, NQ, 512], F32, name=f"ytil{g}{hh}",
                                    tag=f"ytil{hh}")
                        )
                        denoms.append(
                            p2.tile([4, 512], F32, name=f"den{g}{hh}",
                                    tag=f"den{hh}")
                        )
                    for cq in range(NQ):
                        psY = [
                            psY_pool.tile([HD + 1, 512], F32, name=f"psY{hh}",
                                          tag=f"psY{hh}")
                            for hh in range(2)
                        ]
                        njt = 4 * cq + 4

                        def emit_S(j):
                            """S^T matmuls for both heads of the pair at k-tile
                            j; returns the exp'd tiles."""
                            es = []
                            for hh in range(2):
                                poff = 64 * hh
                                psS = psS_pool.tile(
                                    [128, 512], F32, name="psS", tag=f"psS{hh}"
                                )
                                first = True
                                if j >= 4 * cq:
                                    nc.tensor.matmul(
                                        psS,
                                        lhsT=ident_sb,
                                        rhs=maskneg_sb[:, j - 4 * cq, :],
                                        start=True,
                                        stop=False,
                                    )
                                    first = False
                                nc.tensor.matmul(
                                    psS,
                                    lhsT=qk_packed[2 + g][
                                        poff:poff + 64, j * 128:(j + 1) * 128],
                                    rhs=qk_packed[g][
                                        poff:poff + 64,
                                        cq * 512:(cq + 1) * 512],
                                    start=first,
                                    stop=True,
                                )
                                eS = p2e.tile([128, 512], F32R, name="eS",
                                              tag=f"eS{hh}")
                                nc.scalar.activation(
                                    out=eS, in_=psS,
                                    func=mybir.ActivationFunctionType.Exp,
                                    scale=inv_sqrt_hd,
                                )
                                es.append(eS)
                            return es

                        def emit_PV(j, es):
                            for hh in range(2):
                                h = 2 * g + hh      # local head in 0..3
                                nc.tensor.matmul(
                                    psY[hh],
                                    lhsT=vtil[:, j, h, :],
                                    rhs=es[hh],
                                    start=(j == 0),
                                    stop=(j == njt - 1),
                                )

                        # software pipeline: PV(j) is emitted after S(j+1) so
                        # the PE never waits on the exp of the tile it just
                        # produced.
                        prev = emit_S(0)
                        for j in range(1, njt):
                            cur = emit_S(j)
                            emit_PV(j - 1, prev)
                            prev = cur
                        emit_PV(njt - 1, prev)

                        for hh in range(2):
                            # copy numerators + denominator row to SBUF, then
                            # gather the denominator row (cross-partition) by
                            # SBUF->SBUF DMA into the per-head collector.
                            nc.vector.tensor_copy(
                                out=ytils[hh][:, cq, :],
                                in_=psY[hh][:, :],
                            )
                            nc.sync.dma_start(
                                out=denoms[hh][cq:cq + 1, :],
                                in_=ytils[hh][HD:HD + 1, cq, :],
                            )
                    # normalize pair
                    for hh in range(2):
                        rec = p2d.tile([4, 512], F32, name="rec", tag="rec")
                        nc.vector.reciprocal(rec, denoms[hh])
                        recr = p2d.tile([4, 512], F32R, name="recr", tag="recr")
                        nc.vector.tensor_copy(out=recr, in_=rec)
                        for cq in range(NQ):
                            psB = psB_pool.tile([64, 512], F32, name="psB",
                                                tag="psB")
                            nc.tensor.matmul(
                                psB,
                                lhsT=sel4_sb[:, cq * 64:(cq + 1) * 64],
                                rhs=recr,
                                start=True,
                                stop=True,
                            )
                            if hh == 0:
                                nc.vector.tensor_mul(
                                    ynorm[g][0:64, cq * 512:(cq + 1) * 512],
                                    ytils[hh][0:64, cq, :],
                                    psB,
                                )
                            else:
                                fix = p2d.tile([64, 512], F32R, name="fix",
                                               tag="fix")
                                nc.vector.tensor_mul(
                                    fix,
                                    ytils[hh][0:64, cq, :],
                                    psB,
                                )
                                nc.sync.dma_start(
                                    out=ynorm[g][64:128,
                                                 cq * 512:(cq + 1) * 512],
                                    in_=fix,
                                )

            # ================= Phase 3: projection =================
            with (
                tc.tile_pool(name="p3", bufs=1) as p3,
                tc.tile_pool(name="psO", bufs=4, space="PSUM") as psO_pool,
            ):
                wp_sb = p3.tile([128, 2, C], F32R, name="wp_sb")
                nc.sync.dma_start(
                    out=wp_sb, in_=wp.rearrange("(gg p) n -> p gg n", p=128)
                )
                for tt in range(NT):
                    for nck in range(2):
                        pso = psO_pool.tile([128, 512], F32, name="pso",
                                            tag="pso")
                        for g in range(2):
                            nc.tensor.matmul(
                                pso,
                                lhsT=ynorm[g][:, tt * 128:(tt + 1) * 128],
                                rhs=wp_sb[:, g, nck * 512:(nck + 1) * 512],
                                start=(g == 0),
                                stop=(g == 1),
                            )
                        ost = p3.tile([128, 512], F32, name="ost", tag="ost",
                                      bufs=4)
                        nc.scalar.copy(out=ost, in_=pso)
                        nc.sync.dma_start(
                            out=outp[tt * 128:(tt + 1) * 128,
                                     nck * 512:(nck + 1) * 512],
                            in_=ost,
                        )

    nc.finalize()
    return nc


def _prep_in_maps(x, w_attn, w_proj):
    x = np.asarray(x, dtype=np.float32)
    w_attn = np.asarray(w_attn, dtype=np.float32)
    w_proj = np.asarray(w_proj, dtype=np.float32)

    cos2, sinm2 = _rope_tables_T()
    maskneg = _mask_tiles()
    ident = np.eye(128, dtype=np.float32)
    sel4 = np.zeros((4, 4 * HD), dtype=np.float32)
    for cq in range(4):
        sel4[cq, cq * 64:(cq + 1) * 64] = 1.0

    xTs = [np.ascontiguousarray(x[b].T) for b in range(B)]
    in_maps = []
    for core in range(N_CORES):
        b = core // 4
        hbase = (core % 4) * HPC
        # wqk columns: [q_h0|q_h1, q_h2|q_h3, k_h0|k_h1, k_h2|k_h3]
        qcols = w_attn[:, hbase * HD:(hbase + HPC) * HD]
        kcols = w_attn[:, C + hbase * HD:C + (hbase + HPC) * HD]
        vcols = w_attn[:, 2 * C + hbase * HD:2 * C + (hbase + HPC) * HD]
        wqk = np.ascontiguousarray(np.concatenate([qcols, kcols], axis=1))
        wv = np.ascontiguousarray(vcols)
        wp = np.ascontiguousarray(w_proj[hbase * HD:(hbase + HPC) * HD, :])
        in_maps.append({
            "xT": xTs[b],
            "wqk": wqk,
            "wv": wv,
            "wp": wp,
            "cos2": cos2,
            "sinm2": sinm2,
            "maskneg": maskneg,
            "ident": ident,
            "sel4": sel4,
        })
    return in_maps


def _get_runner():
    """Build the SPMD jitted callable once and cache it (mirrors
    bass2jax.run_bass_via_pjrt, but reusable across kernel() calls)."""
    if "runner" in _CACHE:
        return _CACHE["runner"]

    import jax
    from jax.sharding import Mesh, PartitionSpec
    try:
        from jax.experimental.shard_map import shard_map
    except ImportError:
        from jax.shard_map import shard_map  # newer jax
    import concourse.mybir as _mybir
    from concourse import bass2jax

    nc = build_nc()
    bass2jax.install_neuronx_cc_hook()

    partition_name = (
        nc.partition_id_tensor.name if nc.partition_id_tensor else None
    )
    in_names, out_names, out_avals, zero_outs = [], [], [], []
    for alloc in nc.m.functions[0].allocations:
        if not isinstance(alloc, _mybir.MemoryLocationSet):
            continue
        name = alloc.memorylocations[0].name
        if alloc.kind == "ExternalInput":
            if name != partition_name:
                in_names.append(name)
        elif alloc.kind == "ExternalOutput":
            shape = tuple(alloc.tensor_shape)
            dtype = _mybir.dt.np(alloc.dtype)
            out_names.append(name)
            out_avals.append(jax.core.ShapedArray(shape, dtype))
            zero_outs.append(np.zeros(shape, dtype))
    n_params = len(in_names)
    all_names = list(in_names) + list(out_names)
    if partition_name is not None:
        all_names.append(partition_name)
    donate = tuple(range(n_params, n_params + len(out_names)))

    def _body(*args):
        operands = list(args)
        if partition_name is not None:
            operands.append(bass2jax.partition_id_tensor())
        outs = bass2jax._bass_exec_p.bind(
            *operands,
            out_avals=tuple(out_avals),
            in_names=tuple(all_names),
            out_names=tuple(out_names),
            lowering_input_output_aliases=(),
            sim_require_finite=True,
            sim_require_nnan=True,
            nc=nc,
        )
        return tuple(outs)

    devices = jax.devices()[:N_CORES]
    mesh = Mesh(np.asarray(devices), ("core",))
    in_specs = (PartitionSpec("core"),) * (n_params + len(out_names))
    out_specs = (PartitionSpec("core"),) * len(out_names)
    sharded = jax.jit(
        shard_map(_body, mesh=mesh, in_specs=in_specs, out_specs=out_specs,
                  check_rep=False),
        donate_argnums=donate,
        keep_unused=True,
    )

    def run(in_maps):
        concat_in = [
            np.concatenate([np.asarray(in_maps[c][nm]) for c in range(N_CORES)],
                           axis=0)
            for nm in in_names
        ]
        concat_zeros = [
            np.zeros((N_CORES * z.shape[0], *z.shape[1:]), z.dtype)
            for z in zero_outs
        ]
        out_arrs = sharded(*concat_in, *concat_zeros)
        return [
            {
                nm: np.asarray(out_arrs[i]).reshape(
                    N_CORES, *out_avals[i].shape)[c]
                for i, nm in enumerate(out_names)
            }
            for c in range(N_CORES)
        ]

    _CACHE["runner"] = run
    return run


def kernel(x, w_attn, w_proj, n_head):
    assert int(n_head) == NH
    x = np.asarray(x, dtype=np.float32)
    assert x.shape == (B, T, C), x.shape

    run = _get_runner()
    in_maps = _prep_in_maps(x, np.asarray(w_attn), np.asarray(w_proj))
    results = run(in_maps)
    out = np.zeros((B, T, C), dtype=np.float32)
    for core in range(N_CORES):
        out[core // 4] += results[core]["outp"]
    return out


if __name__ == "__main__":
    rng = np.random.default_rng(0)
    x = rng.standard_normal((B, T, C)).astype(np.float32)
    wa = (rng.standard_normal((C, 3 * C)) / np.sqrt(C)).astype(np.float32)
    wpj = (rng.standard_normal((C, C)) / np.sqrt(C)).astype(np.float32)
    y = kernel(x, wa, wpj, NH)
    print("kernel ran, out:", y.shape, y.dtype, float(np.abs(y).mean()))
